# revision 7
# baseline (speedup 1.0000x reference)
"""Local (windowed) attention kernel for Trainium2, sequence-parallel over 8 NeuronCores.

Design notes:
  - [j,i] scores per k-tile (one N<=256 matmul per (t, head) serves both query
    windows), exp batched per same-parity head pair on scalar, mask on vector.
  - pt-stationary attn@v -> po[i, d] with ones-column denominator; [128, 8]
    reciprocal; per-position tensor_scalar normalize; PE transpose; bf16 E.
  - Same-parity head pairs per PSUM bank (different PE row-quadrants writing
    one bank concurrently wedge the device).
  - Contiguous block-major DMA layouts issued from both the sync and scalar
    queues; 256-token lead-in blocks so the tensor engine starts early.
  - Attention-phase work (scores / attn@v+normalize / transpose+projection) is
    queued as closures and drained one item between successive projection
    chains of the following block, so cross-engine latency hides under GEMM
    work and the tensor queue never starves at block boundaries.
  - gpsimd tensor ops wedge the device (no PSUM port engine, NRT status 101)
    and fp8 DoubleRow runs at 1 cycle/row here (no gain) - both avoided.
"""

import sys

sys.path.insert(0, "/opt/trn_rl_repo")

import numpy as np
import ml_dtypes

import concourse.bass as bass
import concourse.mybir as mybir
import concourse.tile as tile
from concourse import bacc
from concourse.bass_utils import run_bass_kernel_spmd

BF16 = mybir.dt.bfloat16
F32 = mybir.dt.float32

N = 16384
DIM = 1024
HEADS = 8
DHEAD = 64
WSZ = 128
NCORES = 8
R = N // NCORES            # 2048 own rows per core
T = R + WSZ                # 2176 rows incl. halo
NW = R // WSZ              # 16 own windows
NT = NW + 1                # 17 k-tiles incl. halo tile 0
DK = DIM // 128            # 8 contraction chunks
P = 128
SCALE = DHEAD ** -0.5

# Position p in pt/o_sb holds head HEAD_ORDER[p]; pairs (2g, 2g+1) share the
# same 64-partition offset so their score matmuls into one PSUM bank use the
# same PE row quadrant (different quadrants would run concurrently and
# conflict on the bank).
HEAD_ORDER = [0, 2, 4, 6, 1, 3, 5, 7]

# token blocks: two short lead-in blocks so B starts as soon as the first
# 0.5MB of x lands, then 512-wide blocks
BLOCKS = [(0, 256), (256, 256), (512, 512), (1024, 512), (1536, 512), (2048, 128)]
XOFF = []                   # byte-column offset of each block in the packed x
_off = 0
for _b0, _bw in BLOCKS:
    XOFF.append(_off)
    _off += DK * _bw

_CACHE = {}


def _score_cols(t):
    """Query-column range [i0, i1) served by k-tile t."""
    i0 = max(t * P, P)
    i1 = min((t + 2) * P, T)
    return i0, i1


def _build():
    nc = bacc.Bacc()
    # packed x: per partition, blocks back to back, each block chunk-major
    xT_d = nc.declare_dram_parameter("xT", [P, DK * T], BF16, isOutput=False)
    # wq packed m-major [P, 8, DK, 128] (q,k) and wv [P, DK, 512]
    w_d = nc.declare_dram_parameter("wqkv", [P, 8, DK, P], BF16, isOutput=False)
    wv_d = nc.declare_dram_parameter("wv", [P, DK, 512], BF16, isOutput=False)
    # consts: mask4 [0:512] | ident [512:640] | wout [640:4736]
    consts_d = nc.declare_dram_parameter("consts", [P, 4736], BF16, isOutput=False)
    out_d = nc.declare_dram_parameter("out", [R, DIM], BF16, isOutput=True)

    with tile.TileContext(nc) as tc:
        with (
            tc.tile_pool(name="pers", bufs=1) as pers,
            tc.tile_pool(name="ptp", bufs=4) as ptp,
            tc.tile_pool(name="osb", bufs=3) as osb,
            tc.tile_pool(name="r8p", bufs=2) as r8p,
            tc.tile_pool(name="attp", bufs=2) as attp,
            tc.tile_pool(name="oep", bufs=3) as oep,
            tc.tile_pool(name="ps512", bufs=2, space="PSUM") as ps512,
            tc.tile_pool(name="pscore", bufs=3, space="PSUM") as pscore,
            tc.tile_pool(name="po4p", bufs=2, space="PSUM") as po4p,
            tc.tile_pool(name="ptrp", bufs=1, space="PSUM") as ptrp,
        ):
            # ---- persistent inputs ---------------------------------------
            xs = [pers.tile([P, DK, bw], BF16, tag=f"x{b}", name=f"x{b}")
                  for b, (b0, bw) in enumerate(BLOCKS)]
            wq = pers.tile([P, 8, DK, P], BF16, tag="wq")
            wv = pers.tile([P, DK, 512], BF16, tag="wv")
            consts = pers.tile([P, 4736], BF16, tag="consts")
            mask4 = consts[:, 0:512].rearrange("p (g w) -> p g w", g=4)
            ident = consts[:, 512:640]
            # x blocks stream on the sync queue; weights + consts issue in
            # parallel from the scalar queue, m-chain granularity
            for m in range(4):
                nc.scalar.dma_start(wq[:, m], w_d[:, m])
            nc.sync.dma_start(
                xs[0].rearrange("p k w -> p (k w)"),
                xT_d[:, XOFF[0]:XOFF[0] + DK * BLOCKS[0][1]])
            for m in range(4, 8):
                nc.scalar.dma_start(wq[:, m], w_d[:, m])
            nc.sync.dma_start(
                xs[1].rearrange("p k w -> p (k w)"),
                xT_d[:, XOFF[1]:XOFF[1] + DK * BLOCKS[1][1]])
            nc.scalar.dma_start(wv[:], wv_d[:])
            nc.scalar.dma_start(consts[:], consts_d[:])
            for b in range(2, len(BLOCKS)):
                nc.sync.dma_start(
                    xs[b].rearrange("p k w -> p (k w)"),
                    xT_d[:, XOFF[b]:XOFF[b] + DK * BLOCKS[b][1]])

            qk = [pers.tile([P, T], BF16, tag=f"qk{m}", name=f"qk{m}") for m in range(8)]
            v_sb = [pers.tile([P, HEADS, DHEAD + 1], BF16, tag=f"v{t}", name=f"v{t}")
                    for t in range(NT)]
            pt = {}          # t -> pt tile [P, HEADS, 256] bf16

            evac_ctr = [0]

            def evac(dst, src):
                # alternate PSUM evacuations between vector and scalar engines
                evac_ctr[0] += 1
                if evac_ctr[0] % 2 == 0:
                    nc.vector.tensor_copy(dst, src)
                else:
                    nc.scalar.copy(dst, src)

            qinit = [False]

            def emit_B_chain(b, b0, bw, m):
                lo = P if (b == 0 and m < 4) else 0
                if lo and not qinit[0] and m == 3:
                    # defensively zero the skipped halo-q columns (never read,
                    # but keep SBUF free of stray NaN bit patterns)
                    for mm in range(4):
                        nc.vector.memset(qk[mm][:, 0:P], 0.0)
                    qinit[0] = True
                pq = ps512.tile([P, 512], F32, tag="mm512", name="mm512")
                for kc in range(DK):
                    nc.tensor.matmul(
                        pq[:, :bw - lo],
                        lhsT=wq[:, m, kc, :],
                        rhs=xs[b][:, kc, lo:bw],
                        start=(kc == 0), stop=(kc == DK - 1),
                    )
                evac(qk[m][:, b0 + lo:b0 + bw], pq[:, :bw - lo])

            def emit_C(t, b, b0):
                loc = t * P - b0
                pv = ps512.tile([P, 512], F32, tag="mm512", name="mm512")
                for kc in range(DK):
                    nc.tensor.matmul(
                        pv[:],
                        lhsT=xs[b][:, kc, loc:loc + P],
                        rhs=wv[:, kc, :],
                        start=(kc == 0), stop=(kc == DK - 1),
                    )
                nc.vector.memset(v_sb[t][:, :, DHEAD:DHEAD + 1], 1.0)
                evac(
                    v_sb[t][:, :, 0:DHEAD],
                    pv.rearrange("p (h d) -> p h d", h=HEADS),
                )

            def emit_scores(t):
                i0, i1 = _score_cols(t)
                w = i1 - i0
                ptt = ptp.tile([P, HEADS, 2 * P], BF16, tag="pt", name="pt")
                pt[t] = ptt
                for g in range(4):          # same-parity head pairs
                    psc = pscore.tile([P, 2, 2 * P], F32, tag="psc", name="psc")
                    for hh in range(2):
                        h = HEAD_ORDER[2 * g + hh]
                        mq, off = h // 2, (h % 2) * 64
                        mk = 4 + h // 2
                        nc.tensor.matmul(
                            psc[:, hh, 0:w],
                            lhsT=qk[mk][off:off + 64, t * P:(t + 1) * P],
                            rhs=qk[mq][off:off + 64, i0:i1],
                            start=True, stop=True,
                        )
                    nc.scalar.activation(
                        ptt[:, 2 * g:2 * g + 2, 0:w], psc[:, :, 0:w],
                        mybir.ActivationFunctionType.Exp, scale=SCALE,
                    )
                if t >= 1:
                    # queries of window t see k_t as their own (current) window:
                    # causal mask on the first 128 columns, 4 heads per op
                    for g4 in range(2):
                        nc.vector.tensor_tensor(
                            pt[t][:, 4 * g4:4 * g4 + 4, 0:P],
                            pt[t][:, 4 * g4:4 * g4 + 4, 0:P],
                            mask4,
                            op=mybir.AluOpType.mult,
                        )

            o_of = {}      # tau -> normalized o_sb tile awaiting transpose+E

            def tail_front(tau):
                # attn@v + normalize for query tile tau (T-coords)
                r8 = r8p.tile([P, HEADS], F32, tag="r8", name="r8")
                o_sb = osb.tile([P, HEADS * DHEAD], BF16, tag="o_sb", name="o_sb")
                o_of[tau] = o_sb
                prevoff = 0 if tau == 1 else P
                for gg in range(2):         # position quads
                    po = po4p.tile([P, 4, DHEAD + 1], F32, tag="po4", name="po4")
                    for hl in range(4):
                        pos = 4 * gg + hl
                        h = HEAD_ORDER[pos]
                        nc.tensor.matmul(
                            po[:, hl, :],
                            lhsT=pt[tau - 1][:, pos, prevoff:prevoff + P],
                            rhs=v_sb[tau - 1][:, h, :],
                            start=True, stop=False,
                        )
                        nc.tensor.matmul(
                            po[:, hl, :],
                            lhsT=pt[tau][:, pos, 0:P],
                            rhs=v_sb[tau][:, h, :],
                            start=False, stop=True,
                        )
                    nc.vector.reciprocal(r8[:, 4 * gg:4 * gg + 4], po[:, :, DHEAD])
                    for hl in range(4):
                        pos = 4 * gg + hl
                        nc.vector.tensor_scalar_mul(
                            o_sb[:, pos * DHEAD:(pos + 1) * DHEAD],
                            po[:, hl, 0:DHEAD],
                            r8[:, pos:pos + 1],
                        )
            def tail_back(tau):
                # transpose + output projection for query tile tau
                o_sb = o_of.pop(tau)
                ptr = ptrp.tile([P, 4, P], BF16, tag="ptr", name="ptr")
                for mch in range(4):
                    nc.tensor.transpose(
                        ptr[:, mch, :], o_sb[:, mch * P:(mch + 1) * P], ident,
                    )
                att = attp.tile([P, 4, P], BF16, tag="att", name="att")
                evac(att[:], ptr[:])
                for nf in range(2):
                    pf = ps512.tile([P, 512], F32, tag="mm512", name="mm512")
                    for m in range(4):
                        nc.tensor.matmul(
                            pf[:],
                            lhsT=att[:, m, :],
                            rhs=consts[:, 640 + m * DIM + nf * 512:
                                       640 + m * DIM + (nf + 1) * 512],
                            start=(m == 0), stop=(m == 3),
                        )
                    oe = oep.tile([P, 512], BF16, tag="oe", name="oe")
                    evac(oe[:], pf[:])
                    nc.sync.dma_start(
                        out_d[(tau - 1) * P:tau * P, nf * 512:(nf + 1) * 512], oe[:],
                    )

            # ---- interleaved emission ------------------------------------
            # D work (scores, tail fronts/backs) is queued as closures and
            # drained one item between successive B/C chains of the NEXT
            # block, so cross-engine latency always hides under GEMM work.
            from collections import deque
            dq = deque()

            def pump():
                if dq:
                    dq.popleft()()

            t_done = 0
            tau_f = [1]
            tau_b = [1]
            for b, (b0, bw) in enumerate(BLOCKS):
                for m in range(8):
                    emit_B_chain(b, b0, bw, m)
                    pump()
                    pump()
                for t in range(b0 // P, (b0 + bw) // P):
                    emit_C(t, b, b0)
                    pump()
                    pump()
                t_hi = -1
                for t in range(t_done, NT):
                    if min((t + 2) * P, T) <= b0 + bw:
                        t_hi = t
                while t_done <= t_hi:
                    dq.append(lambda t=t_done: emit_scores(t))
                    t_done += 1
                    while tau_f[0] <= t_done - 2:
                        dq.append(lambda tau=tau_f[0]: tail_front(tau))
                        tau_f[0] += 1
                        if tau_b[0] <= tau_f[0] - 2:
                            dq.append(lambda tau=tau_b[0]: tail_back(tau))
                            tau_b[0] += 1
            while t_done < NT:
                dq.append(lambda t=t_done: emit_scores(t))
                t_done += 1
            while tau_f[0] <= NW:
                dq.append(lambda tau=tau_f[0]: tail_front(tau))
                tau_f[0] += 1
                if tau_b[0] <= tau_f[0] - 2:
                    dq.append(lambda tau=tau_b[0]: tail_back(tau))
                    tau_b[0] += 1
            while tau_b[0] <= NW:
                dq.append(lambda tau=tau_b[0]: tail_back(tau))
                tau_b[0] += 1
            while dq:
                pump()

    nc.compile()
    return nc


def _get_nc():
    if "nc" not in _CACHE:
        _CACHE["nc"] = _build()
    return _CACHE["nc"]


def _prep_inputs(x, w_qkv, w_out):
    """Host-side shard + layout prep. Returns per-core input maps."""
    x = np.asarray(x, dtype=np.float32)
    w_qkv = np.asarray(w_qkv, dtype=np.float32)
    w_out = np.asarray(w_out, dtype=np.float32)

    # wq m-major [P, 8, DK, 128] over q,k cols; wv [P, DK, 512]
    wq = np.ascontiguousarray(
        w_qkv[:, 0:1024].reshape(DK, P, 8, P).transpose(1, 2, 0, 3)
    ).astype(ml_dtypes.bfloat16)
    wv = np.ascontiguousarray(
        w_qkv[:, 1024:1536].reshape(DK, P, 512).transpose(1, 0, 2)
    ).astype(ml_dtypes.bfloat16)
    w_out_perm = w_out.reshape(HEADS, DHEAD, DIM)[HEAD_ORDER].reshape(
        HEADS * DHEAD, DIM)
    wo = np.ascontiguousarray(
        w_out_perm.reshape(4, P, DIM).transpose(1, 0, 2)
    ).astype(ml_dtypes.bfloat16)
    mask1 = np.triu(np.ones((P, P), dtype=np.float32))       # [j, i]: j <= i
    mask4 = np.broadcast_to(mask1[:, None, :], (P, 4, P)).reshape(P, 512)
    ident = np.eye(P, dtype=np.float32)
    consts = np.concatenate(
        [mask4, ident, wo.reshape(P, 4 * DIM).astype(np.float32)], axis=1
    ).astype(ml_dtypes.bfloat16)

    x_pad = np.concatenate([np.zeros((WSZ, DIM), np.float32), x], axis=0)
    in_maps = []
    for c in range(NCORES):
        x_sh = x_pad[c * R:c * R + T]                        # (2176, 1024)
        xT = np.ascontiguousarray(
            x_sh.T.reshape(DK, P, T).transpose(1, 0, 2)      # [P, DK, T]
        )
        xpack = np.empty((P, DK * T), dtype=ml_dtypes.bfloat16)
        for bi, (b0, bw) in enumerate(BLOCKS):
            xpack[:, XOFF[bi]:XOFF[bi] + DK * bw] = (
                xT[:, :, b0:b0 + bw].reshape(P, DK * bw).astype(ml_dtypes.bfloat16)
            )
        in_maps.append({
            "xT": xpack,
            "wqkv": wq,
            "wv": wv,
            "consts": consts,
        })
    return in_maps


def kernel(x, w_qkv, w_out, b_out):
    b_out = np.asarray(b_out, dtype=np.float32)
    in_maps = _prep_inputs(x, w_qkv, w_out)
    nc = _get_nc()
    res = run_bass_kernel_spmd(nc, in_maps, core_ids=list(range(NCORES)))
    out = np.concatenate(
        [res.results[c]["out"].astype(np.float32) for c in range(NCORES)], axis=0
    )
    return out + b_out[None, :]
